# revision 3
# baseline (speedup 1.0000x reference)
"""Multi-head attention (B=8, S=1024, D=1024, H=16, dh=64) on 8 trn2 cores.

Sharding: data-parallel over batch - one batch element per NeuronCore.

On top of that: host pre-chunks all six inputs into per-partition-
contiguous layouts so each load is one ~128-descriptor DMA (issue cost
~0.7us instead of ~2.3us), loads are ordered by true consumption
deadline (the j1 half of the V projection feeds only heads 8-15, i.e.
pairs 4-7, so it streams last), the exp table is warmed during the DMA
lead-in, projection chains are emitted j0-first so the first exp fires
~15us earlier, outputs are stored as one batched DMA per head pair, and
the last pair is head-split so its AV overlaps the final exps.

v2 vs baseline: the two heads of a pair score CONCURRENTLY on the PE.
Each scores PSUM tile [128, 1024] holds head0's 512 sq-columns in bank A
(written by a 64-row-tile matmul at array rows 0:63) and head1's in bank B
(array rows 64:127). The two matmuls run in different PE row groups and
different PSUM banks, so the hardware overlaps them fully (verified by
microbenchmark: pair cadence == single-matmul cadence). exp() then covers
both heads in one [128, 2x512] activation into a combined E buffer
es_pair [128, 8, 2048] = (sk_tile, head*1024 + sq).
"""

import numpy as np

S = 1024   # sequence length (queries == keys)
D = 1024   # model dim
F = 1024   # heads * head_dim
H = 16
DH = 64
P = 128
NCORES = 8
C = 68     # per-head column stride in the V buffer (64 vals + 1 ones + pad)

_cached_nc = None


def _build_nc():
    import concourse.tile as tile
    from concourse import bacc, mybir

    f32 = mybir.dt.float32
    f16 = mybir.dt.float16
    Exp = mybir.ActivationFunctionType.Exp

    nc = bacc.Bacc("TRN2", target_bir_lowering=False, debug=False,
                   num_devices=NCORES)

    # host pre-chunks every tensor to per-partition-contiguous layouts so
    # each load is ~128 DMA descriptors (one run per partition):
    #   rhs-role (xq, xk, wv):  [P, j(2), dc(8), 512]
    #   lhsT-role (wq, wk, xv): [P, m(8), dc(8), 128]
    xq_t = nc.dram_tensor("xq_t", [P, D * S // P], f16,
                          kind="ExternalInput").ap()
    xk_t = nc.dram_tensor("xk_t", [P, D * S // P], f16,
                          kind="ExternalInput").ap()
    xv_t = nc.dram_tensor("xv_t", [P, D * S // P], f16,
                          kind="ExternalInput").ap()
    wq = nc.dram_tensor("wq", [P, D * F // P], f16, kind="ExternalInput").ap()
    wk = nc.dram_tensor("wk", [P, D * F // P], f16, kind="ExternalInput").ap()
    wv = nc.dram_tensor("wv", [P, D * F // P], f16, kind="ExternalInput").ap()
    out = nc.dram_tensor("out", [S, F], f32, kind="ExternalOutput").ap()

    KD = D // P   # 8 contraction tiles
    out_v = out.rearrange("(sm p) f -> p sm f", p=P)

    def rhs_view(t):
        return t.rearrange("p (j dc s) -> p j dc s", j=2, dc=KD)

    def lhs_view(t):
        return t.rearrange("p (m dc c) -> p m dc c", m=8, dc=KD)

    with tile.TileContext(nc) as tc:
        with (
            tc.tile_pool(name="persist", bufs=1) as persist,
            tc.tile_pool(name="inputs", bufs=1) as inputs,
            tc.tile_pool(name="e_pool", bufs=2) as e_pool,
            tc.tile_pool(name="kq_ring", bufs=4) as kq_ring,
            tc.tile_pool(name="pout", bufs=2) as pout,
            tc.tile_pool(name="small", bufs=4) as small,
            tc.tile_pool(name="pp_ps", bufs=2, space="PSUM") as pp_ps,
            tc.tile_pool(name="s_ps", bufs=2, space="PSUM") as s_ps,
            tc.tile_pool(name="o_ps", bufs=2, space="PSUM") as o_ps,
        ):
            # warm the exp activation table while input DMAs stream
            # (self-contained on the scalar engine: no cross-engine wait)
            warm = small.tile([P, 8], f32, tag="warm")
            nc.scalar.memzero(warm)
            nc.scalar.activation(warm, warm, Exp)

            v65 = persist.tile([P, S // P, H * C], f16, tag="v65")
            v_heads = v65.rearrange("p s (h c) -> p s h c", c=C)

            def rhs_tile(tag):
                return inputs.tile([P, 2, KD, 512], f16, tag=tag, name=tag)

            def lhs_tile(tag):
                return inputs.tile([P, 8, KD, P], f16, tag=tag, name=tag)

            xk_sb = rhs_tile("kx")
            wk_sb = lhs_tile("kw")
            xq_sb = rhs_tile("qx")
            wq_sb = lhs_tile("qw")
            xv_sb = lhs_tile("vx")
            wv_sb = rhs_tile("vw")

            # batched contiguous loads, ordered by true consumption
            # deadline; v_proj's j1 half (heads 8-15) is not needed until
            # pair 4, so wv-j1 loads last
            nc.sync.dma_start(xk_sb[:, 0], rhs_view(xk_t)[:, 0])
            nc.sync.dma_start(wk_sb[:, 0], lhs_view(wk)[:, 0])
            nc.sync.dma_start(xq_sb[:, 0], rhs_view(xq_t)[:, 0])
            nc.sync.dma_start(wq_sb[:, 0], lhs_view(wq)[:, 0])
            nc.sync.dma_start(xq_sb[:, 1], rhs_view(xq_t)[:, 1])
            nc.sync.dma_start(xk_sb[:, 1], rhs_view(xk_t)[:, 1])
            nc.sync.dma_start(wk_sb[:, 1], lhs_view(wk)[:, 1])
            nc.sync.dma_start(wq_sb[:, 1], lhs_view(wq)[:, 1])
            nc.sync.dma_start(xv_sb[:, 0:4], lhs_view(xv_t)[:, 0:4])
            nc.sync.dma_start(wv_sb[:, 0], rhs_view(wv)[:, 0])
            nc.sync.dma_start(wk_sb[:, 2], lhs_view(wk)[:, 2])
            nc.sync.dma_start(wq_sb[:, 2], lhs_view(wq)[:, 2])
            nc.sync.dma_start(xv_sb[:, 4:8], lhs_view(xv_t)[:, 4:8])
            nc.sync.dma_start(wk_sb[:, 3], lhs_view(wk)[:, 3])
            nc.sync.dma_start(wq_sb[:, 3], lhs_view(wq)[:, 3])
            nc.sync.dma_start(wk_sb[:, 4:8], lhs_view(wk)[:, 4:8])
            nc.sync.dma_start(wq_sb[:, 4:8], lhs_view(wq)[:, 4:8])
            nc.sync.dma_start(wv_sb[:, 1], rhs_view(wv)[:, 1])
            nc.gpsimd.memset(v_heads[:, :, :, DH:DH + 1], 1.0)

            def proj_chain(psum_pool, ptag, lhs_sb, rhs_sb, m, j):
                ps = psum_pool.tile([P, 512], f32, tag=ptag)
                for dc in range(KD):
                    nc.tensor.matmul(
                        ps[:, :],
                        lhsT=lhs_sb[:, m, dc, :],
                        rhs=rhs_sb[:, j, dc, :],
                        start=(dc == 0), stop=(dc == KD - 1),
                    )
                return ps

            def kq_proj(m):
                kc = kq_ring.tile([P, S], f16, tag="kc")
                qc = kq_ring.tile([P, S], f16, tag="qc")
                for j in range(2):
                    for lhs_sb, rhs_sb, dst in ((wk_sb, xk_sb, kc),
                                                (wq_sb, xq_sb, qc)):
                        ps = proj_chain(pp_ps, "pp", lhs_sb, rhs_sb, m, j)
                        nc.vector.tensor_copy(dst[:, j * 512:(j + 1) * 512],
                                              ps[:, :])
                return kc, qc

            def v_proj_j(j, ms):
                for m in ms:
                    ps = proj_chain(o_ps, "o", xv_sb, wv_sb, m, j)
                    src = ps.rearrange("p (h c) -> p h c", c=DH)
                    dst = v_heads[:, m, j * 8:(j + 1) * 8, 0:DH]
                    nc.vector.tensor_copy(dst, src)

            def scores_exp(kc, qc):
                # es_pair free layout: [sk_tile(8), head(2)*1024 + sq]
                es = e_pool.tile([P, S // P, 2 * S], f16, tag="e")
                esv = es.rearrange("p s (h j q) -> p s h j q", h=2, j=2)
                for skm in range(8):
                    for j in range(2):
                        ps = s_ps.tile([P, 1024], f32, tag="s")
                        for i in range(2):
                            b0 = i * DH
                            nc.tensor.matmul(
                                ps[:, i * 512:(i + 1) * 512],
                                lhsT=kc[b0:b0 + DH, skm * P:(skm + 1) * P],
                                rhs=qc[b0:b0 + DH, j * 512:(j + 1) * 512],
                                start=True, stop=True,
                            )
                        psv = ps.rearrange("p (h q) -> p h q", h=2)
                        nc.scalar.activation(esv[:, skm, :, j, :], psv,
                                             Exp, scale=0.125)
                return es

            def scores_exp_head(kc, qc, es, i):
                # tail variant: one head's full scores, serial matmuls,
                # exp per (skm) so the head's E completes independently
                b0 = i * DH
                for skm in range(8):
                    ps = s_ps.tile([P, 1024], f32, tag="s")
                    for j in range(2):
                        nc.tensor.matmul(
                            ps[:, j * 512:(j + 1) * 512],
                            lhsT=kc[b0:b0 + DH, skm * P:(skm + 1) * P],
                            rhs=qc[b0:b0 + DH, j * 512:(j + 1) * 512],
                            start=True, stop=True,
                        )
                    nc.scalar.activation(es[:, skm, i * S:(i + 1) * S], ps,
                                         Exp, scale=0.125)

            def av_norm(hp, es, heads=(0, 1), po=None, store="batched"):
                if po is None:
                    po = pout.tile([P, S // P, P], f32, tag="po",
                                   name=f"po{min(heads)}")
                chains = [(sqm, i) for sqm in range(8) for i in heads]
                for qs in range(0, len(chains), 4):
                    quad = chains[qs:qs + 4]
                    nq = len(quad)
                    # 4 chains share one PSUM bank: the first matmul's
                    # start=True clears the whole bank's has_written bits, so
                    # each later chain's first matmul overwrites fresh
                    ps_o = o_ps.tile([P, 512], f32, tag="o")
                    for c, (sqm, i) in enumerate(quad):
                        h = 2 * hp + i
                        for kt in range(8):
                            nc.tensor.matmul(
                                ps_o[:, c * P:c * P + DH + 1],
                                lhsT=es[:, kt,
                                        i * S + sqm * P:i * S + (sqm + 1) * P],
                                rhs=v65[:, kt, h * C:h * C + DH + 1],
                                start=(c == 0 and kt == 0),
                                stop=(c == nq - 1 and kt == 7),
                                skip_group_check=True,
                            )
                    st = small.tile([P, 4, DH + 1], f32, tag="st")
                    src_v = ps_o.rearrange("p (c x) -> p c x", x=P)
                    nc.vector.tensor_copy(st[:, 0:nq, :],
                                          src_v[:, 0:nq, 0:DH + 1])
                    rt = small.tile([P, 4, 1], f32, tag="r")
                    nc.vector.reciprocal(rt[:, 0:nq, :],
                                         st[:, 0:nq, DH:DH + 1])
                    for c, (sqm, i) in enumerate(quad):
                        nc.vector.tensor_scalar_mul(
                            po[:, sqm, i * DH:(i + 1) * DH],
                            st[:, c, 0:DH], rt[:, c, :])
                if store == "batched":
                    nc.sync.dma_start(out_v[:, :, hp * P:(hp + 1) * P], po)
                elif store == "per_sqm":
                    for sqm in range(8):
                        nc.sync.dma_start(
                            out[sqm * P:(sqm + 1) * P, hp * P:(hp + 1) * P],
                            po[:, sqm, :])
                return po

            # software pipeline over head pairs; kq_proj(p+1) is emitted
            # before av_norm(p-1) so the next pair's scores inputs are ready
            # the moment the exp stream drains
            e_prev = None
            LAST = H // 2 - 1
            kc, qc = kq_proj(0)
            for hp in range(LAST):
                es = scores_exp(kc, qc)
                kc_n, qc_n = kq_proj(hp + 1)
                if hp < 4:
                    # j0 half (heads 0-7) feeds av(0..3); j1 feeds av(4..7)
                    v_proj_j(hp // 2, range((hp % 2) * 4, (hp % 2) * 4 + 4))
                if e_prev is not None:
                    av_norm(hp - 1, e_prev)
                e_prev = es
                kc, qc = kc_n, qc_n
            # last pair head-granular: head 0's AV overlaps head 1's exp
            es = e_pool.tile([P, S // P, 2 * S], f16, tag="e")
            scores_exp_head(kc, qc, es, 0)
            av_norm(LAST - 1, e_prev)
            scores_exp_head(kc, qc, es, 1)
            po_last = av_norm(LAST, es, heads=(0,), store=None)
            av_norm(LAST, es, heads=(1,), po=po_last, store="batched")

    nc.compile()
    return nc


def _get_nc():
    global _cached_nc
    if _cached_nc is None:
        _cached_nc = _build_nc()
    return _cached_nc


def _rhs_chunk(a_t):
    # [D, S] -> [P, j*dc*512]: per-partition contiguous j-major chunks
    f = np.ascontiguousarray(a_t).astype(np.float16)
    return np.ascontiguousarray(
        f.reshape(8, P, 2, 512).transpose(1, 2, 0, 3).reshape(P, -1))


def _lhs_chunk(a_t):
    # [D, F] -> [P, m*dc*128]: per-partition contiguous m-major chunks
    f = np.ascontiguousarray(a_t).astype(np.float16)
    return np.ascontiguousarray(
        f.reshape(8, P, 8, P).transpose(1, 2, 0, 3).reshape(P, -1))


def _in_maps(queries, keys, values, Wq, Wk, Wv):
    wqb = _lhs_chunk(Wq)
    wkb = _lhs_chunk(Wk)
    wvb = _rhs_chunk(Wv)
    maps = []
    for b in range(NCORES):
        maps.append({
            "xq_t": _rhs_chunk(queries[b].T),
            "xk_t": _rhs_chunk(keys[b].T),
            "xv_t": _lhs_chunk(values[b].T),
            "wq": wqb, "wk": wkb, "wv": wvb,
        })
    return maps


def kernel(queries, keys, values, Wq, Wk, Wv, _trace=False):
    from concourse import bass_utils

    queries = np.asarray(queries)
    keys = np.asarray(keys)
    values = np.asarray(values)
    Wq, Wk, Wv = np.asarray(Wq), np.asarray(Wk), np.asarray(Wv)
    nc = _get_nc()
    maps = _in_maps(queries, keys, values, Wq, Wk, Wv)
    res = bass_utils.run_bass_kernel_spmd(
        nc, maps, core_ids=list(range(NCORES)), trace=_trace)
    out = np.stack([res.results[b]["out"] for b in range(NCORES)])
    if _trace:
        kernel.last_results = res
    return out
